# revision 1
# baseline (speedup 1.0000x reference)
"""Trainium2 Bass kernel for EpiLinear (epinet + prior-ensemble MLP).

Strategy (per spec sharding hint: data-parallel over batch, params replicated):
  - Shard B=2048 across 8 cores (256 rows each).
  - Key algebraic restructure: the epinet input is concat([xf, z]) where
    xf = concat(x, feature) is broadcast over the n=8 z-samples. So
      h = relu(epinet_inp @ Wep1 + b) = relu(A[b] + Bz[b,n] + b)
    with A = xf @ Wep1[:2048] computed ONCE per batch row (8x fewer FLOPs
    than the naive [B*n, 2080] GEMM) and Bz = z @ Wep1[2048:] tiny.
  - All activations are kept feature-on-partitions (transposed) so every
    GEMM contraction maps to the PE partition axis with no on-chip
    transposes; the host pre-transposes x/feature/z (cheap numpy prep).
  - Prior ensemble (32 tiny MLPs 1024->5->5->1) is flattened into dense
    GEMMs via host-built block-diagonal weight matrices.
  - Final reduction over the 32 noise dims is a partition-group sum done
    with one matmul against a group-selection matrix.
  - Heavy tensors (w1, xfT, wp1, zT, w2, hidden acts) travel/compute in
    bf16 (fp32 PSUM accumulation); small/sensitive paths use float32r.
  - Bz matmuls (K=32) are row-packed 4x and out2 matmuls (M=32) are
    col-packed 4x via tile_position so they run concurrently in the PE.
  - Per hid-tile software pipeline: PE does [Bz(m), A(m)] while DVE/GPSIMD
    run h(m-1) = relu(A + b + Bz), so elementwise work hides under GEMMs.
  - Small parameters ride in one packed DMA; w1/xfT are host-swizzled
    into SBUF layout so every bulk DMA is fully contiguous on both sides.
"""

import time

import numpy as np
import ml_dtypes

import concourse.bacc as bacc
import concourse.mybir as mybir
import concourse.tile as tile
from concourse.bass_utils import run_bass_kernel_spmd

F32 = mybir.dt.float32
F32R = mybir.dt.float32r
BF16 = mybir.dt.bfloat16
RELU = mybir.ActivationFunctionType.Relu
COPY = mybir.ActivationFunctionType.Copy
ADD = mybir.AluOpType.add
MULT = mybir.AluOpType.mult

USE_BF16 = True
DT = BF16 if USE_BF16 else F32R
NPDT = ml_dtypes.bfloat16 if USE_BF16 else np.float32

N_CORES = 8
B, N_Z, ND, SD, HD = 2048, 8, 32, 1024, 1024
EH = 512                  # epinet hidden
XF = SD + HD              # 2048 concat(x, feature) features
BL = B // N_CORES         # 256 batch rows per core
R = BL * N_Z              # 2048 epinet rows per core (r = n*BL + b, n-major)
PHF = 160                 # 32 ensembles * 5 prior hidden, flattened
KT = XF // 128            # 16 k-tiles over xf features
MT = EH // 128            # 4 hid tiles of epinet hidden
RC = R // 512             # 4 chunks of 512 epinet rows

# packed bf16 param block column offsets: w1z4 | w2 | wp2a | wp2b | wp3a |
# wp3b | wp1
PB_W1Z, PB_W2, PB_W2A, PB_W2B = 0, 512, 640, 800
PB_W3A, PB_W3B, PB_WP1, PB_COLS = 960, 992, 1024, 2304

_CACHE = {}


def _build():
    nc = bacc.Bacc("TRN2", target_bir_lowering=False, debug=False,
                   num_devices=N_CORES)
    f = lambda name, shape, dt: nc.dram_tensor(name, shape, dt, kind="ExternalInput").ap()
    xfT = f("xfT", [128, KT * BL], DT)  # xf.T slice, SBUF-layout swizzled
    w1 = f("w1", [128, MT, KT * 128], DT)  # Wep1[:2048] SBUF-layout swizzled
    zT4 = f("zT4", [128, R], DT)        # z^T (r n-major) replicated 4x
    zp = f("zp", [128, 512], F32)       # z^T packed into 4 partition strips
    packb = f("packb", [128, PB_COLS], DT)   # all small bf16 params
    bias = f("bias", [128, 9], F32)     # packed per-partition biases
    sel4 = f("sel4", [128, 4], F32R)    # group-sum selection matrix
    out = nc.dram_tensor("out", [RC, 512], F32, kind="ExternalOutput").ap()

    with tile.TileContext(nc) as tc:
        with (
            tc.tile_pool(name="const", bufs=1) as cp,
            tc.tile_pool(name="work", bufs=1) as wk,
            tc.tile_pool(name="tmp", bufs=4) as tp,
            tc.tile_pool(name="ps_a", bufs=2, space="PSUM") as ps_a,
            tc.tile_pool(name="ps_bz", bufs=5, space="PSUM") as ps_bz,
            tc.tile_pool(name="ps_o2", bufs=1, space="PSUM") as ps_o2,
        ):
            # ---- SBUF tiles -------------------------------------------------
            xfT_sb = cp.tile([128, KT * BL], DT)      # [p, (k b)]
            w1_sb = cp.tile([128, MT * KT * 128], DT)  # [p, (m k h)]
            zT4_sb = cp.tile([128, R], DT)
            pk_sb = cp.tile([128, PB_COLS], DT)
            zp_sb = cp.tile([128, 512], F32)
            bias_sb = cp.tile([128, 9], F32)
            sel4_sb = cp.tile([128, 4], F32R)

            A_sb = wk.tile([128, MT * BL], F32)       # A^T (no bias), [p, (m b)]
            h_sb = [wk.tile([128, R], DT, name=f"h{m}") for m in range(MT)]
            h1a_sb = wk.tile([128, BL], DT)
            h1b_sb = wk.tile([32, BL], DT)
            h2a_sb = wk.tile([128, BL], DT)
            h2b_sb = wk.tile([32, BL], DT)
            prep_sb = wk.tile([128, 512], F32)        # prior out, strip+col replicated
            g_sb = wk.tile([128, 512], F32)
            gm_sb = wk.tile([128, 512], F32R)
            out_sb = wk.tile([RC, 512], F32)

            x3 = xfT_sb[:].rearrange("p (k b) -> p k b", b=BL)
            w13 = w1_sb[:].rearrange("p (m k h) -> p m k h", m=MT, h=128)
            w1z4_v = pk_sb[:, PB_W1Z:PB_W1Z + EH]
            w23 = pk_sb[:, PB_W2:PB_W2A].rearrange("p (k o) -> p k o", o=ND)
            wp2a_v = pk_sb[:, PB_W2A:PB_W2B]
            wp2b_v = pk_sb[0:32, PB_W2B:PB_W3A]
            wp3a_v = pk_sb[:, PB_W3A:PB_W3B]
            wp3b_v = pk_sb[0:32, PB_W3B:PB_WP1]
            wp13 = pk_sb[:, PB_WP1:PB_COLS].rearrange("p (k g) -> p k g", g=PHF)

            # ---- DMAs: small/early on scalar queue, bulk on sync queue -----
            nc.scalar.dma_start(pk_sb[:, 0:PB_W2A], packb[:, 0:PB_W2A])
            nc.scalar.dma_start(zT4_sb[:], zT4[:])
            nc.scalar.dma_start(pk_sb[:, PB_W2A:], packb[:, PB_W2A:])
            nc.gpsimd.dma_start(bias_sb[:], bias[:])
            nc.gpsimd.dma_start(sel4_sb[:], sel4[:])
            nc.gpsimd.dma_start(zp_sb[:], zp[:])

            HKT = KT * 128 // 2
            def _xf(c):
                nc.sync.dma_start(
                    xfT_sb[:, 4 * c * BL:(4 * c + 4) * BL],
                    xfT[:, 4 * c * BL:(4 * c + 4) * BL])
            def _w1(c, half):
                nc.sync.dma_start(
                    w1_sb[:, (c * 2 + half) * HKT:(c * 2 + half + 1) * HKT],
                    w1[:, c, half * HKT:(half + 1) * HKT])
            # w1[m0] early (PE can start), then the rest of xf (so A(m0)
            # COMPLETES early and the h pipeline drains), then w1 m1..m3
            # half-chunks which pace the remaining stages.
            _xf(0); _w1(0, 0); _w1(0, 1); _xf(1); _xf(2); _xf(3)
            for c in range(1, 4):
                _w1(c, 0); _w1(c, 1)

            # ---- epinet L1, software-pipelined per hid-tile m:
            #   PE: [Bz(m) -> A(m)]  ||  DVE/GPSIMD: h(m-1) = relu(A+b+Bz)
            # Bz(m) runs while the w1[m] DMA chunk is still in flight.
            for m in range(MT):
                psz = [ps_bz.tile([128, 512], F32, tag="pz", name=f"pz{m}_{rc}")
                       for rc in range(RC)]
                for rc in range(RC):
                    nc.tensor.matmul(
                        psz[rc][:, :],
                        w1z4_v[32 * rc:32 * rc + 32, 128 * m:128 * m + 128],
                        zT4_sb[32 * rc:32 * rc + 32, 512 * rc:512 * rc + 512],
                        start=True, stop=True, tile_position=(32 * rc, 0))
                psA = ps_a.tile([128, BL], F32, tag="pa", name=f"pA{m}")
                for k in range(KT):
                    nc.tensor.matmul(
                        psA[:, :], w13[:, m, k, :],
                        x3[:, k, :], start=(k == 0), stop=(k == KT - 1))
                nc.scalar.activation(A_sb[:, BL * m:BL * (m + 1)], psA[:, :],
                                     COPY)
                Ab = A_sb[:, BL * m:BL * (m + 1)].unsqueeze(1).broadcast_to(
                    (128, 2, BL))
                for rc in range(RC):
                    t = tp.tile([128, 512], DT, tag="t")
                    nc.vector.scalar_tensor_tensor(
                        t[:].rearrange("p (a b) -> p a b", a=2),
                        psz[rc][:, :].rearrange("p (a b) -> p a b", a=2),
                        bias_sb[:, m:m + 1], Ab, op0=ADD, op1=ADD)
                    nc.gpsimd.tensor_scalar_max(
                        h_sb[m][:, 512 * rc:512 * rc + 512], t[:], 0.0)

            # ---- prior ensemble: h1 = relu(x @ wp1 + b) --------------------
            ps1 = []
            for m, (mp, m0) in enumerate([(128, 0), (32, 128)]):
                ps = ps_a.tile([128, BL], F32, tag="pa", name=f"pp1_{m}")
                for k in range(8):  # x = first 1024 features of xfT
                    nc.tensor.matmul(
                        ps[0:mp, :], wp13[:, k, m0:m0 + mp], x3[:, k, :],
                        start=(k == 0), stop=(k == 7))
                ps1.append(ps)
            nc.scalar.activation(h1a_sb[:], ps1[0][0:128, :], RELU,
                                 bias=bias_sb[:, 4:5])
            nc.scalar.activation(h1b_sb[:], ps1[1][0:32, :], RELU,
                                 bias=bias_sb[0:32, 5:6])

            # h2 = relu(h1 @ wp2 + b)   (block-diag dense)
            ps2 = []
            for m, (mp, m0) in enumerate([(128, 0), (32, 128)]):
                ps = ps_a.tile([128, BL], F32, tag="pa", name=f"pp2_{m}")
                nc.tensor.matmul(ps[0:mp, :], wp2a_v[:, m0:m0 + mp],
                                 h1a_sb[:], start=True, stop=False)
                nc.tensor.matmul(ps[0:mp, :], wp2b_v[:, m0:m0 + mp],
                                 h1b_sb[:], start=False, stop=True)
                ps2.append(ps)
            nc.scalar.activation(h2a_sb[:], ps2[0][0:128, :], RELU,
                                 bias=bias_sb[:, 6:7])
            nc.scalar.activation(h2b_sb[:], ps2[1][0:32, :], RELU,
                                 bias=bias_sb[0:32, 7:8])

            # p = h2 @ wp3   -> [32 ensembles, BL]
            psp = ps_a.tile([128, BL], F32, tag="pa", name="ppp")
            nc.tensor.matmul(psp[0:32, :], wp3a_v[:], h2a_sb[:],
                             start=True, stop=False)
            nc.tensor.matmul(psp[0:32, :], wp3b_v[:], h2b_sb[:],
                             start=False, stop=True)
            # p replicated into 4 partition strips x 2 column copies
            pb = psp[0:32, :].unsqueeze(1).broadcast_to((32, 2, BL))
            for c in range(RC):
                nc.scalar.activation(
                    prep_sb[32 * c:32 * c + 32, :].rearrange(
                        "p (a b) -> p a b", a=2), pb, COPY)

            # ---- epinet L2 col-packed 4x: out2^T[32rc+k, q] ----------------
            pso = ps_o2.tile([128, 512], F32, tag="po", name="po")
            for k in range(MT):
                for rc in range(RC):
                    nc.tensor.matmul(
                        pso[32 * rc:32 * rc + 32, :], w23[:, k, :],
                        h_sb[k][:, 512 * rc:512 * rc + 512],
                        start=(k == 0), stop=(k == MT - 1),
                        tile_position=(0, 32 * rc), skip_group_check=True)
            # g = (out2 + (bep2 + bp3)) + p ; gm = g * z
            nc.vector.scalar_tensor_tensor(
                g_sb[:], pso[:, :], bias_sb[:, 8:9], prep_sb[:],
                op0=ADD, op1=ADD)
            nc.vector.tensor_tensor(gm_sb[:], g_sb[:], zp_sb[:], op=MULT)
            # partition-group sum over the 32 noise dims
            psS = ps_o2.tile([128, 512], F32, tag="po", name="psS")
            nc.tensor.matmul(psS[0:RC, :], sel4_sb[:], gm_sb[:].bitcast(F32R),
                             start=True, stop=True)
            nc.scalar.activation(out_sb[:], psS[0:RC, :], COPY)
            nc.sync.dma_start(out[:], out_sb[:])

    nc.compile()
    return nc


def _prep(x, feature, z, Wep1, bep1, Wep2, bep2, Wp1, bp1, Wp2, bp2, Wp3, bp3):
    """Host-side weight/layout prep shared across cores."""
    c32 = lambda a: np.ascontiguousarray(np.asarray(a, dtype=np.float32))
    xfT = np.ascontiguousarray(
        np.concatenate([x, feature], axis=1).T.astype(NPDT))  # [XF, B]
    # swizzle w1 into SBUF layout [p, m, (k h)]
    w1 = np.ascontiguousarray(
        np.asarray(Wep1, np.float32)[:XF].astype(NPDT)
        .reshape(KT, 128, MT, 128).transpose(1, 2, 0, 3)
        .reshape(128, MT, KT * 128))

    packb = np.zeros((128, PB_COLS), NPDT)
    packb[:, PB_W1Z:PB_W2] = np.tile(np.asarray(Wep1, np.float32)[XF:], (4, 1))
    packb[:, PB_W2:PB_W2A] = (np.asarray(Wep2, np.float32)
                              .reshape(4, 128, ND).transpose(1, 0, 2)
                              .reshape(128, 4 * ND))
    wp2 = np.zeros((PHF, PHF), np.float32)
    wp3 = np.zeros((PHF, ND), np.float32)
    for e in range(ND):
        wp2[5 * e:5 * e + 5, 5 * e:5 * e + 5] = Wp2[e]
        wp3[5 * e:5 * e + 5, e] = np.asarray(Wp3)[e, :, 0]
    packb[:, PB_W2A:PB_W2B] = wp2[0:128]
    packb[0:32, PB_W2B:PB_W3A] = wp2[128:160]
    packb[:, PB_W3A:PB_W3B] = wp3[0:128]
    packb[0:32, PB_W3B:PB_WP1] = wp3[128:160]
    packb[:, PB_WP1:PB_COLS] = (np.asarray(Wp1, np.float32)
                                .transpose(1, 0, 2).reshape(SD, PHF)
                                .reshape(8, 128, PHF).transpose(1, 0, 2)
                                .reshape(128, 8 * PHF))

    bias = np.zeros((128, 9), np.float32)
    bias[:, 0:4] = np.asarray(bep1, np.float32).reshape(4, 128).T
    bp1f = np.asarray(bp1, np.float32).reshape(PHF)
    bp2f = np.asarray(bp2, np.float32).reshape(PHF)
    bias[:, 4] = bp1f[:128]
    bias[:32, 5] = bp1f[128:]
    bias[:, 6] = bp2f[:128]
    bias[:32, 7] = bp2f[128:]
    bias[:, 8] = np.tile(np.asarray(bep2, np.float32)
                         + np.asarray(bp3, np.float32)[:, 0], 4)
    sel4 = np.zeros((128, 4), np.float32)
    sel4[np.arange(128), np.arange(128) // 32] = 1.0
    shared = dict(w1=w1, packb=packb, bias=bias, sel4=sel4)
    in_maps = []
    for c in range(N_CORES):
        sl = slice(c * BL, (c + 1) * BL)
        zTf = np.asarray(z)[sl].transpose(1, 0, 2).reshape(R, ND).T  # [32, R]
        zpk = c32(np.ascontiguousarray(zTf).reshape(ND, RC, 512)
                  .transpose(1, 0, 2).reshape(128, 512))
        m = dict(shared)
        m["xfT"] = np.ascontiguousarray(
            xfT[:, sl].reshape(KT, 128, BL).transpose(1, 0, 2)
            .reshape(128, KT * BL))
        m["zT4"] = np.ascontiguousarray(np.tile(zTf, (4, 1)).astype(NPDT))
        m["zp"] = zpk
        in_maps.append(m)
    return in_maps


def kernel(**inputs):
    if "nc" not in _CACHE:
        _CACHE["nc"] = _build()
    nc = _CACHE["nc"]
    in_maps = _prep(**inputs)
    last_err = None
    for _attempt in range(3):
        try:
            res = run_bass_kernel_spmd(nc, in_maps, list(range(N_CORES)))
            full = np.empty((B, N_Z, 1), np.float32)
            for c in range(N_CORES):
                S = np.asarray(res.results[c]["out"]).reshape(R)
                full[c * BL:(c + 1) * BL, :, 0] = S.reshape(N_Z, BL).T
            return full
        except Exception as e:  # transient device/transfer hiccups
            last_err = e
            time.sleep(5.0 * (_attempt + 1))
    raise last_err



# revision 71
# speedup vs baseline: 1.2629x; 1.2629x over previous
"""Trainium2 Bass kernel for EpiLinear (epinet + prior-ensemble MLP).

Strategy (data-parallel over batch, params replicated, b-major row order
r = b*8 + n per core):
  - A = xf @ Wep1[:2048] computed once per batch row in TRANSPOSED layout
    (psum [b, h]) so its rows can be re-used as matmul LHS directly.
  - The broadcast-add of A over the 8 z-samples is FUSED into the Bz
    matmul: lhs = [W1z ; A^T-slice] (K=96), rhs = [z^T ; indicator].
    One 512-row matmul per (hid-tile, row-chunk) yields A+Bz in PSUM;
    relu+bias is a single fused Activation/DVE op per chunk.
  - Epinet L2 runs transposed (out[r-part, 32]) so it charges 32 rows
    per matmul instead of 512.  The prior output p and all output-side
    biases are accumulated into the same PSUM via tiny K=16 indicator
    matmuls against p^T (built with two PE transposes).
  - Final out = rowwise dot(out2+p+bias, z): one tensor_tensor multiply
    and one grouped free-axis reduce, split in halves to shorten the
    tail; output DMA'd as [128, 16] and unshuffled on host.
  - Prior ensemble (32 MLPs 1024->5->5->1) flattened into dense GEMMs
    via host-built block-diagonal weights (as before).
  - All heavy tensors travel in bf16; w1 is chunked per 128-hid group
    so the A/Bz pipeline overlaps the DMA stream.
"""

import time

import numpy as np
import ml_dtypes

import concourse.bacc as bacc
import concourse.mybir as mybir
import concourse.tile as tile
from concourse.bass_utils import run_bass_kernel_spmd

F32 = mybir.dt.float32
BF16 = mybir.dt.bfloat16
RELU = mybir.ActivationFunctionType.Relu
COPY = mybir.ActivationFunctionType.Copy
ADD = mybir.AluOpType.add
MULT = mybir.AluOpType.mult
MAX = mybir.AluOpType.max
AXE_X = mybir.AxisListType.X

NPDT = ml_dtypes.bfloat16

N_CORES = 8
B, N_Z, ND, SD, HD = 2048, 8, 32, 1024, 1024
EH = 512                  # epinet hidden
XF = SD + HD              # 2048 concat(x, feature) features
BL = B // N_CORES         # 256 batch rows per core
R = BL * N_Z              # 2048 epinet rows per core (r = b*8 + n, b-major)
KT = XF // 128            # 16 k-tiles over xf features
NU = EH // 128            # 4 hid-tiles (u == m)
NRC = 4                   # 512-row chunks (64 b each)
NT = R // 128             # 16 row-tiles for L2
PHF = 160                 # 32 ensembles * 5 prior hidden, flattened

# packed bf16 param block column offsets (wp1 first so its DMA can lead)
PB_WP1 = 0
PB_W2, PB_W2A, PB_W2B, PB_W3A, PB_W3B = 1280, 1408, 1568, 1728, 1760
PB_IND, PB_EYE, PB_COLS = 1792, 1920, 1952

_CACHE = {}


def _build():
    nc = bacc.Bacc("TRN2", target_bir_lowering=False, debug=False,
                   num_devices=N_CORES)
    f = lambda name, shape, dt: nc.dram_tensor(name, shape, dt, kind="ExternalInput").ap()
    xfT = f("xfT", [128, KT * BL], BF16)      # xf^T swizzled [p, (k b)]
    w1 = f("w1", [128, NU * KT * 128], BF16)  # Wep1[:2048] [p, (u k c)]
    zT = f("zT", [32, R], BF16)               # z^T (b-major)
    indq = f("indq", [64, 512], BF16)         # indicator, one 512 block
    w1zq = f("w1zq", [32, 512], BF16)         # Wep1[2048:], one 512 block
    zrt = f("zrt", [128, NT * ND], BF16)      # z in [r-part, (t nd)]
    packb = f("packb", [128, PB_COLS], BF16)  # small bf16 params
    bias = f("bias", [128, 9], F32)           # packed per-partition biases
    out = nc.dram_tensor("out", [128, NT], F32, kind="ExternalOutput").ap()

    with tile.TileContext(nc) as tc:
        with (
            tc.tile_pool(name="const", bufs=1) as cp,
            tc.tile_pool(name="work", bufs=1) as wk,
            tc.tile_pool(name="ps_a", bufs=2, space="PSUM") as ps_a,
            tc.tile_pool(name="ps_z", bufs=4, space="PSUM") as ps_z,
            tc.tile_pool(name="ps_o", bufs=1, space="PSUM") as ps_o,
            tc.tile_pool(name="ps_p", bufs=1, space="PSUM") as ps_p,
        ):
            # ---- SBUF tiles ------------------------------------------------
            xfT_sb = cp.tile([128, KT * BL], BF16)
            w1_sb = cp.tile([128, NU * KT * 128], BF16)
            zind_sb = cp.tile([96, R], BF16)
            lhsA_sb = cp.tile([96, R], BF16)   # [W1z-tiled ; A^T copies]
            zrt_sb = cp.tile([128, NT * ND], BF16)
            pk_sb = cp.tile([128, PB_COLS], BF16)
            bias_sb = cp.tile([128, 9], F32)

            h_sb = [wk.tile([128, R], BF16, name=f"h{m}") for m in range(NU)]
            h1a_sb = wk.tile([128, BL], BF16)
            h1b_sb = wk.tile([32, BL], BF16)
            h2a_sb = wk.tile([128, BL], BF16)
            h2b_sb = wk.tile([33, BL], BF16)   # row 32 = ones (bias lane)
            p_sb = wk.tile([32, BL], BF16)
            pT2_sb = wk.tile([16, NT * ND], BF16)   # p^T, [b%16, (b//16, nd)]
            gm_sb = wk.tile([128, NT * ND], BF16)
            out_sb = wk.tile([128, NT], F32)
            dum_sb = wk.tile([1, 1], F32)

            x3 = xfT_sb[:].rearrange("p (k b) -> p k b", b=BL)
            wuv = w1_sb[:].rearrange("p (u k c) -> p u k c", u=NU, c=128)
            w23 = pk_sb[:, PB_W2:PB_W2A].rearrange("p (m o) -> p m o", o=ND)
            wp2a_v = pk_sb[:, PB_W2A:PB_W2B]
            wp2b_v = pk_sb[0:32, PB_W2B:PB_W3A]
            wp3a_v = pk_sb[:, PB_W3A:PB_W3B]
            wp3b_v = pk_sb[0:33, PB_W3B:PB_IND]
            wp13 = pk_sb[:, PB_WP1:PB_W2].rearrange("p (k g) -> p k g", g=PHF)
            ind16_v = pk_sb[0:16, PB_IND:PB_IND + 128]
            eye32_v = pk_sb[0:32, PB_EYE:PB_EYE + 32]

            # ---- DMAs ------------------------------------------------------
            # Pool queue (SWDGE): tiny transfers first so the SP prolog
            # chunks win the shared DMA engines early.
            nc.gpsimd.dma_start(xfT_sb[:, 0:BL], xfT[:, 0:BL])
            nc.gpsimd.dma_start(zind_sb[0:64, 0:512], indq[:])
            nc.gpsimd.dma_start(lhsA_sb[64:96, 0:512], w1zq[:])
            nc.gpsimd.dma_start(pk_sb[:, PB_WP1:PB_W2], packb[:, PB_WP1:PB_W2])
            nc.gpsimd.dma_start(bias_sb[:], bias[:])
            nc.gpsimd.dma_start(zind_sb[64:96, :], zT[:])
            nc.gpsimd.dma_start(pk_sb[:, PB_W2:], packb[:, PB_W2:])
            # on-chip replication of the constant blocks (replaces 0.33 MB
            # of DMA): Act handles the indicator, Pool the W1z strip
            for rc in range(1, NRC):
                nc.scalar.activation(zind_sb[0:64, 512 * rc:512 * rc + 512],
                                     zind_sb[0:64, 0:512], COPY)
                nc.gpsimd.tensor_copy(lhsA_sb[64:96, 512 * rc:512 * rc + 512],
                                      lhsA_sb[64:96, 0:512])
            # SP queue (HWDGE): xf and w1 interleaved, k-progressive for u0,
            # whole-u blocks for u1/u2, u3 split so its tail chain is short.
            def _xf(k0, k1):
                nc.sync.dma_start(xfT_sb[:, k0 * BL:k1 * BL],
                                  xfT[:, k0 * BL:k1 * BL])
            def _w1(c0, c1):
                nc.sync.dma_start(w1_sb[:, c0 * 128:c1 * 128],
                                  w1[:, c0 * 128:c1 * 128])
            _w1(0, 1)
            _w1(1, 4); _xf(1, 4)
            _w1(4, 8); _xf(4, 8)
            _w1(8, 16); _xf(8, 12)
            _xf(12, 16)
            _w1(16, 32)              # u1
            _w1(32, 48)              # u2
            _w1(48, 56); _w1(56, 64)  # u3 halves
            # zrt rides last on the busy queue: needed only at the combine
            nc.sync.dma_start(zrt_sb[:], zrt[:])

            # preload the activation table while DMAs stream
            nc.scalar.activation(dum_sb[:], bias_sb[0:1, 0:1], RELU)
            # ones lane feeding the output-bias row of the L3 matmul
            nc.vector.memset(h2b_sb[32:33, :], 1.0)

            # ---- helpers ---------------------------------------------------
            psA = [ps_a.tile([128, 512], F32, tag="pa", name=f"pA{bt}")
                   for bt in range(2)]

            def a_mm(u, bt, k):
                nc.tensor.matmul(
                    psA[bt][:, 128 * u:128 * u + 128],
                    x3[:, k, 128 * bt:128 * bt + 128],
                    wuv[:, u, k, :],
                    start=(k == 0), stop=(k == KT - 1),
                    skip_group_check=True)

            def a_u(u, with_prior=False, ps1=None):
                # k-paced for u0 (DMA streaming); bt-major k-halves otherwise
                # so the first batch-half's psA completes early for copies
                if with_prior:
                    for k in range(KT):
                        for bt in range(2):
                            a_mm(u, bt, k)
                        if k < 8:
                            for i, (mp, m0) in enumerate([(128, 0), (32, 128)]):
                                nc.tensor.matmul(
                                    ps1[i][0:mp, :], wp13[:, k, m0:m0 + mp],
                                    x3[:, k, :], start=(k == 0 and i == 0),
                                    stop=(k == 7 and i == 1),
                                    skip_group_check=True)
                else:
                    for bt in range(2):
                        for k in range(KT):
                            a_mm(u, bt, k)

            at_n = [0]

            def at_copies_bt(u, bt):
                for rh in range(2):
                    src = psA[bt][64 * rh:64 * rh + 64,
                                  128 * u:128 * u + 128]
                    dst = lhsA_sb[0:64,
                                  128 * (4 * (2 * bt + rh) + u):
                                  128 * (4 * (2 * bt + rh) + u) + 128]
                    nc.vector.tensor_copy(dst, src)

            def at_copies(u):
                # A^T slices [64, 128] -> lhsA rows 0:64; the last block's
                # copies ride on Act so DVE is free for the tail relus
                for bt in range(2):
                    for rh in range(2):
                        src = psA[bt][64 * rh:64 * rh + 64,
                                      128 * u:128 * u + 128]
                        dst = lhsA_sb[0:64,
                                      128 * (4 * (2 * bt + rh) + u):
                                      128 * (4 * (2 * bt + rh) + u) + 128]
                        nc.vector.tensor_copy(dst, src)
                        at_n[0] += 1

            relu_n = [0]

            def fz(m, rcs):
                for rc in rcs:
                    psz = ps_z.tile([128, 512], F32, tag="pz",
                                    name=f"pz{m}_{rc}")
                    nc.tensor.matmul(
                        psz[:, :],
                        lhsA_sb[0:96,
                                128 * (4 * rc + m):128 * (4 * rc + m) + 128],
                        zind_sb[0:96, 512 * rc:512 * rc + 512],
                        start=True, stop=True)
                    dst = h_sb[m][:, 512 * rc:512 * rc + 512]
                    if (rc % 2 == 0) if m == NU - 1 else \
                       (True if m == NU - 2 else relu_n[0] % 2 == 0):
                        nc.scalar.activation(dst, psz[:, :], RELU,
                                             bias=bias_sb[:, m:m + 1])
                    else:
                        nc.vector.tensor_scalar(dst, psz[:, :],
                                                bias_sb[:, m:m + 1], 0.0,
                                                op0=ADD, op1=MAX)
                    relu_n[0] += 1

            def l2(ts, ms):
                for t in ts:
                    for m in ms:
                        nc.tensor.matmul(
                            pso_h[t // 8][:, 32 * (t % 8):32 * (t % 8) + 32],
                            h_sb[m][:, 128 * t:128 * t + 128],
                            w23[:, m, :], start=False,
                            stop=(m == NU - 1 and t % 8 == 7),
                            skip_group_check=True)

            # ---- A(u0) + prior L1 -----------------------------------------
            # both prior m-tiles share one psum bank: the [32]-tile opens
            # via the bank's pending-zero (start=False after the big start)
            ps1t = ps_p.tile([128, 512], F32, tag="pp", name="pp1")
            ps1 = [ps1t[:, 0:BL], ps1t[:, BL:2 * BL]]
            a_u(0, with_prior=True, ps1=ps1)
            at_copies(0)
            nc.scalar.activation(h1a_sb[:], ps1[0][0:128, :], RELU,
                                 bias=bias_sb[:, 4:5])
            nc.scalar.activation(h1b_sb[:], ps1[1][0:32, :], RELU,
                                 bias=bias_sb[0:32, 5:6])

            # ---- A(u1); prior L2 ------------------------------------------
            a_u(1)
            at_copies(1)
            ps2t = ps_p.tile([128, 512], F32, tag="pp", name="pp2")
            ps2 = [ps2t[:, 0:BL], ps2t[:, BL:2 * BL]]
            for i, (mp, m0) in enumerate([(128, 0), (32, 128)]):
                ps = ps2[i]
                nc.tensor.matmul(ps[0:mp, :], wp2a_v[:, m0:m0 + mp],
                                 h1a_sb[:], start=(i == 0), stop=False,
                                 skip_group_check=True)
                nc.tensor.matmul(ps[0:mp, :], wp2b_v[:, m0:m0 + mp],
                                 h1b_sb[:], start=False, stop=(i == 1),
                                 skip_group_check=True)
            nc.scalar.activation(h2a_sb[:], ps2[0][0:128, :], RELU,
                                 bias=bias_sb[:, 6:7])
            nc.scalar.activation(h2b_sb[0:32, :], ps2[1][0:32, :], RELU,
                                 bias=bias_sb[0:32, 7:8])

            # ---- FZ(m0); prior L3 + p -------------------------------------
            fz(0, range(NRC))
            psp = ps_p.tile([128, BL], F32, tag="pp", name="ppp")
            nc.tensor.matmul(psp[0:32, :], wp3a_v[:], h2a_sb[:],
                             start=True, stop=False)
            nc.tensor.matmul(psp[0:32, :], wp3b_v[:], h2b_sb[:],
                             start=False, stop=True)
            nc.scalar.activation(p_sb[:], psp[0:32, :], COPY)

            # ---- A(u2); p transpose ---------------------------------------
            a_u(2)
            at_copies(2)
            psT = ps_p.tile([16, NT * ND], BF16, tag="pp", name="ppT")
            for c in range(NT):
                nc.tensor.transpose(psT[:, 32 * c:32 * c + 32],
                                    p_sb[0:32, 16 * c:16 * c + 16], eye32_v)
            nc.scalar.activation(pT2_sb[:], psT[:, :], COPY)

            # ---- FZ(m1); pso init with p + bias ---------------------------
            fz(1, range(NRC))
            # two pso half-tiles in separate banks: combine reads of one
            # half never anti-block L2 writes of the other
            pso_h = [ps_o.tile([128, 256], F32, tag="po", name="po0"),
                     ps_p.tile([128, 256], F32, tag="pp", name="po1")]
            for hh in range(2):
                nc.tensor.matmul(pso_h[hh][:, :], ind16_v,
                                 pT2_sb[0:16, 256 * hh:256 * hh + 256],
                                 start=True, stop=False,
                                 skip_group_check=True)

            # ---- A(u3); FZ(m2, m3); L2; quartered combine -----------------
            fz(2, range(NRC))
            for bt in range(2):
                for k in range(KT):
                    a_mm(3, bt, k)
                at_copies_bt(3, bt)
            fz(3, [0, 1])
            l2(range(0, 4), range(NU))

            def half(q):
                c0, c1 = 256 * q, 256 * q + 256
                nc.vector.tensor_tensor(gm_sb[:, c0:c1], pso_h[q][:, :],
                                        zrt_sb[:, c0:c1], op=MULT)
                nc.vector.tensor_reduce(
                    out_sb[:, 8 * q:8 * q + 8].rearrange(
                        "p (t o) -> p t o", o=1),
                    gm_sb[:, c0:c1].rearrange("p (t o) -> p t o", o=ND),
                    AXE_X, ADD)
                nc.sync.dma_start(out[:, 8 * q:8 * q + 8],
                                  out_sb[:, 8 * q:8 * q + 8])

            fz(3, [2])
            l2(range(4, 8), range(NU))
            half(0)
            fz(3, [3])
            l2(range(8, 12), range(NU))
            l2(range(12, 16), range(NU))
            half(1)

    nc.compile()
    return nc


def _prep(x, feature, z, Wep1, bep1, Wep2, bep2, Wp1, bp1, Wp2, bp2, Wp3, bp3):
    """Host-side weight/layout prep shared across cores (layout/packing only)."""
    Wep1 = np.asarray(Wep1, np.float32)
    xfT = np.ascontiguousarray(
        np.concatenate([x, feature], axis=1).T.astype(NPDT))  # [XF, B]
    # w1 swizzle [p, (u k c)]
    w1 = np.ascontiguousarray(
        Wep1[:XF].astype(NPDT).reshape(KT, 128, NU, 128)
        .transpose(1, 2, 0, 3).reshape(128, NU * KT * 128))
    w1zq = np.ascontiguousarray(Wep1[XF:].astype(NPDT))   # [32, 512]

    ind64 = np.kron(np.eye(64, dtype=np.float32), np.ones((1, 8), np.float32))
    ind16 = np.kron(np.eye(16, dtype=np.float32), np.ones((1, 8), np.float32))

    packb = np.zeros((128, PB_COLS), NPDT)
    packb[:, PB_W2:PB_W2A] = (np.asarray(Wep2, np.float32)
                              .reshape(NU, 128, ND).transpose(1, 0, 2)
                              .reshape(128, NU * ND))
    wp2 = np.zeros((PHF, PHF), np.float32)
    wp3 = np.zeros((PHF, ND), np.float32)
    for e in range(ND):
        wp2[5 * e:5 * e + 5, 5 * e:5 * e + 5] = Wp2[e]
        wp3[5 * e:5 * e + 5, e] = np.asarray(Wp3)[e, :, 0]
    packb[:, PB_W2A:PB_W2B] = wp2[0:128]
    packb[0:32, PB_W2B:PB_W3A] = wp2[128:160]
    packb[:, PB_W3A:PB_W3B] = wp3[0:128]
    packb[0:32, PB_W3B:PB_IND] = wp3[128:160]
    packb[32, PB_W3B:PB_IND] = (np.asarray(bep2, np.float32)
                                + np.asarray(bp3, np.float32)[:, 0])
    packb[:, PB_WP1:PB_W2] = (np.asarray(Wp1, np.float32)
                               .transpose(1, 0, 2).reshape(SD, PHF)
                               .reshape(8, 128, PHF).transpose(1, 0, 2)
                               .reshape(128, 8 * PHF))
    packb[0:16, PB_IND:PB_IND + 128] = ind16
    packb[0:32, PB_EYE:PB_EYE + 32] = np.eye(32, dtype=np.float32)

    bias = np.zeros((128, 9), np.float32)
    bias[:, 0:4] = np.asarray(bep1, np.float32).reshape(NU, 128).T
    bp1f = np.asarray(bp1, np.float32).reshape(PHF)
    bp2f = np.asarray(bp2, np.float32).reshape(PHF)
    bias[:, 4] = bp1f[:128]
    bias[:32, 5] = bp1f[128:]
    bias[:, 6] = bp2f[:128]
    bias[:32, 7] = bp2f[128:]
    bias[:32, 8] = (np.asarray(bep2, np.float32)
                    + np.asarray(bp3, np.float32)[:, 0])

    shared = dict(w1=w1, w1zq=w1zq, packb=packb, bias=bias,
                  indq=np.ascontiguousarray(ind64.astype(NPDT)))
    in_maps = []
    for c in range(N_CORES):
        sl = slice(c * BL, (c + 1) * BL)
        zf = np.asarray(z)[sl].reshape(R, ND)          # r = b*8 + n
        zrt = np.ascontiguousarray(
            zf.astype(NPDT).reshape(NT, 128, ND)
            .transpose(1, 0, 2).reshape(128, NT * ND))
        m = dict(shared)
        m["xfT"] = np.ascontiguousarray(
            xfT[:, sl].reshape(KT, 128, BL).transpose(1, 0, 2)
            .reshape(128, KT * BL))
        m["zT"] = np.ascontiguousarray(zf.T.astype(NPDT))
        m["zrt"] = zrt
        in_maps.append(m)
    return in_maps


def kernel(**inputs):
    if "nc" not in _CACHE:
        _CACHE["nc"] = _build()
    nc = _CACHE["nc"]
    in_maps = _prep(**inputs)
    last_err = None
    for _attempt in range(3):
        try:
            res = run_bass_kernel_spmd(nc, in_maps, list(range(N_CORES)))
            full = np.empty((B, N_Z, 1), np.float32)
            for c in range(N_CORES):
                S = np.asarray(res.results[c]["out"])       # [128, NT]
                full[c * BL:(c + 1) * BL, :, 0] = \
                    S.T.reshape(BL, N_Z)
            return full
        except Exception as e:  # transient device/transfer hiccups
            last_err = e
            time.sleep(5.0 * (_attempt + 1))
    raise last_err


# revision 88
# speedup vs baseline: 1.2899x; 1.0214x over previous
"""Trainium2 Bass kernel for EpiLinear (epinet + prior-ensemble MLP).

Strategy (data-parallel over batch, params replicated, b-major row order
r = b*8 + n per core):
  - A = xf @ Wep1[:2048] computed once per batch row in TRANSPOSED layout
    (psum [b, h]) so its rows can be re-used as matmul LHS directly.
  - The broadcast-add of A over the 8 z-samples is FUSED into the Bz
    matmul: lhs = [A^T-slice ; W1z] (K = 64 + 32), rhs = [indicator ;
    z^T].  One 512-row matmul per (hid-tile, row-chunk) yields A+Bz in
    PSUM; relu+bias is a single fused Activation/DVE op per chunk — no
    separate broadcast-add pass on the vector engines.
  - Epinet L2 runs transposed (out[r-part, 32-free]) so it charges 32
    rows per matmul instead of 512.  The prior output p and all
    output-side biases are broadcast into the same PSUM banks by K=16
    indicator matmuls against p^T (built with 16 tiny PE transposes);
    L2 then accumulates on top.  Two separate pso half-tiles keep the
    combine's PSUM reads from anti-blocking later L2 writes.
  - Final out = rowwise dot(out2+p+bias, z): per half, one
    tensor_tensor multiply and one grouped free-axis reduce on DVE;
    output DMA'd as [128, 16] and unshuffled on host.
  - Prior ensemble (32 MLPs 1024->5->5->1) flattened into dense GEMMs
    via host-built block-diagonal weights; its output bias (bep2+bp3)
    rides a ones-lane appended to h2 in the L3 matmul.
  - All heavy tensors travel in bf16.  w1 is laid out hid-block-major
    and streamed so A(u)/FZ(u) pipeline with the DMA; the last block is
    split so its tail chain is short.  The indicator and W1z constant
    blocks are replicated on-chip (Act/Pool) instead of DMA'd.
  - Engine budget: PE saturated start-to-finish (A 16.4k + FZ 8.2k +
    prior 5.6k + L2 2k + misc rows); relu/copy traffic split between
    Act and DVE per the tuned RELU_ENG map; Pool handles SWDGE DMAs
    and constant replication; SP streams xf/w1/out via HWDGE.
"""

import time

import numpy as np
import ml_dtypes

import concourse.bacc as bacc
import concourse.mybir as mybir
import concourse.tile as tile
from concourse.bass_utils import run_bass_kernel_spmd

F32 = mybir.dt.float32
BF16 = mybir.dt.bfloat16
RELU = mybir.ActivationFunctionType.Relu
COPY = mybir.ActivationFunctionType.Copy
ADD = mybir.AluOpType.add
MULT = mybir.AluOpType.mult
MAX = mybir.AluOpType.max
AXE_X = mybir.AxisListType.X

NPDT = ml_dtypes.bfloat16

N_CORES = 8
B, N_Z, ND, SD, HD = 2048, 8, 32, 1024, 1024
EH = 512                  # epinet hidden
XF = SD + HD              # 2048 concat(x, feature) features
BL = B // N_CORES         # 256 batch rows per core
R = BL * N_Z              # 2048 epinet rows per core (r = b*8 + n, b-major)
KT = XF // 128            # 16 k-tiles over xf features
NU = EH // 128            # 4 hid-tiles (u == m)
NRC = 4                   # 512-row chunks (64 b each)
NT = R // 128             # 16 row-tiles for L2
PHF = 160                 # 32 ensembles * 5 prior hidden, flattened

# packed bf16 param block column offsets (wp1 first so its DMA can lead)
PB_WP1 = 0
PB_W2, PB_W2A, PB_W2B, PB_W3A, PB_W3B = 1280, 1408, 1568, 1728, 1760
PB_IND, PB_EYE, PB_COLS = 1792, 1920, 1952

# relu engine per (m, rc): 'A' = Activation, 'V' = DVE
RELU_ENG = "AVAVAVAVAVAAAVAA"
_CACHE = {}


def _build():
    nc = bacc.Bacc("TRN2", target_bir_lowering=False, debug=False,
                   num_devices=N_CORES)
    f = lambda name, shape, dt: nc.dram_tensor(name, shape, dt, kind="ExternalInput").ap()
    xfT = f("xfT", [128, KT * BL], BF16)      # xf^T swizzled [p, (k b)]
    w1 = f("w1", [128, NU * KT * 128], BF16)  # Wep1[:2048] [p, (u k c)]
    zT = f("zT", [32, R], BF16)               # z^T (b-major)
    indq = f("indq", [64, 512], BF16)         # indicator, one 512 block
    w1zq = f("w1zq", [32, 512], BF16)         # Wep1[2048:], one 512 block
    zrt = f("zrt", [128, NT * ND], BF16)      # z in [r-part, (t nd)]
    packb = f("packb", [128, PB_COLS], BF16)  # small bf16 params
    bias = f("bias", [128, 9], F32)           # packed per-partition biases
    out = nc.dram_tensor("out", [128, NT], F32, kind="ExternalOutput").ap()

    with tile.TileContext(nc) as tc:
        with (
            tc.tile_pool(name="const", bufs=1) as cp,
            tc.tile_pool(name="work", bufs=1) as wk,
            tc.tile_pool(name="ps_a", bufs=2, space="PSUM") as ps_a,
            tc.tile_pool(name="ps_z", bufs=4, space="PSUM") as ps_z,
            tc.tile_pool(name="ps_o", bufs=1, space="PSUM") as ps_o,
            tc.tile_pool(name="ps_p", bufs=1, space="PSUM") as ps_p,
        ):
            # ---- SBUF tiles ------------------------------------------------
            xfT_sb = cp.tile([128, KT * BL], BF16)
            w1_sb = cp.tile([128, NU * KT * 128], BF16)
            zind_sb = cp.tile([96, R], BF16)
            lhsA_sb = cp.tile([96, R], BF16)   # [W1z-tiled ; A^T copies]
            zrt_sb = cp.tile([128, NT * ND], BF16)
            pk_sb = cp.tile([128, PB_COLS], BF16)
            bias_sb = cp.tile([128, 9], F32)

            h_sb = [wk.tile([128, R], BF16, name=f"h{m}") for m in range(NU)]
            h1a_sb = wk.tile([128, BL], BF16)
            h1b_sb = wk.tile([32, BL], BF16)
            h2a_sb = wk.tile([128, BL], BF16)
            h2b_sb = wk.tile([33, BL], BF16)   # row 32 = ones (bias lane)
            p_sb = wk.tile([32, BL], BF16)
            pT2_sb = wk.tile([16, NT * ND], BF16)   # p^T, [b%16, (b//16, nd)]
            gm_sb = wk.tile([128, NT * ND], BF16)
            out_sb = wk.tile([128, NT], F32)
            dum_sb = wk.tile([1, 1], F32)

            x3 = xfT_sb[:].rearrange("p (k b) -> p k b", b=BL)
            wuv = w1_sb[:].rearrange("p (u k c) -> p u k c", u=NU, c=128)
            w23 = pk_sb[:, PB_W2:PB_W2A].rearrange("p (m o) -> p m o", o=ND)
            wp2a_v = pk_sb[:, PB_W2A:PB_W2B]
            wp2b_v = pk_sb[0:32, PB_W2B:PB_W3A]
            wp3a_v = pk_sb[:, PB_W3A:PB_W3B]
            wp3b_v = pk_sb[0:33, PB_W3B:PB_IND]
            wp13 = pk_sb[:, PB_WP1:PB_W2].rearrange("p (k g) -> p k g", g=PHF)
            ind16_v = pk_sb[0:16, PB_IND:PB_IND + 128]
            eye32_v = pk_sb[0:32, PB_EYE:PB_EYE + 32]

            # ---- DMAs ------------------------------------------------------
            # Pool queue (SWDGE): tiny transfers first so the SP prolog
            # chunks win the shared DMA engines early.
            nc.gpsimd.dma_start(xfT_sb[:, 0:BL], xfT[:, 0:BL])
            nc.gpsimd.dma_start(zind_sb[0:64, 0:512], indq[:])
            nc.gpsimd.dma_start(lhsA_sb[64:96, 0:512], w1zq[:])
            nc.gpsimd.dma_start(pk_sb[:, PB_WP1:PB_W2], packb[:, PB_WP1:PB_W2])
            nc.gpsimd.dma_start(bias_sb[:], bias[:])
            nc.gpsimd.dma_start(zind_sb[64:96, :], zT[:])
            nc.gpsimd.dma_start(pk_sb[:, PB_W2:], packb[:, PB_W2:])
            # on-chip replication of the constant blocks (replaces 0.33 MB
            # of DMA): Act handles the indicator, Pool the W1z strip
            for rc in range(1, NRC):
                nc.scalar.activation(zind_sb[0:64, 512 * rc:512 * rc + 512],
                                     zind_sb[0:64, 0:512], COPY)
                nc.gpsimd.tensor_copy(lhsA_sb[64:96, 512 * rc:512 * rc + 512],
                                      lhsA_sb[64:96, 0:512])
            # SP queue (HWDGE): xf and w1 interleaved, k-progressive for u0,
            # whole-u blocks for u1/u2, u3 split so its tail chain is short.
            def _xf(k0, k1):
                nc.sync.dma_start(xfT_sb[:, k0 * BL:k1 * BL],
                                  xfT[:, k0 * BL:k1 * BL])
            def _w1(c0, c1):
                nc.sync.dma_start(w1_sb[:, c0 * 128:c1 * 128],
                                  w1[:, c0 * 128:c1 * 128])
            _w1(0, 1)
            _w1(1, 4); _xf(1, 4)
            _w1(4, 8); _xf(4, 8)
            _w1(8, 16); _xf(8, 12)
            _xf(12, 16)
            _w1(16, 32)              # u1
            _w1(32, 48)              # u2
            _w1(48, 56); _w1(56, 64)  # u3 halves
            # zrt rides last on the busy queue: needed only at the combine
            nc.sync.dma_start(zrt_sb[:], zrt[:])

            # preload the activation table while DMAs stream
            nc.scalar.activation(dum_sb[:], bias_sb[0:1, 0:1], RELU)
            # ones lane feeding the output-bias row of the L3 matmul
            nc.vector.memset(h2b_sb[32:33, :], 1.0)

            # ---- helpers ---------------------------------------------------
            psA = [ps_a.tile([128, 512], F32, tag="pa", name=f"pA{bt}")
                   for bt in range(2)]

            def a_mm(u, bt, k):
                nc.tensor.matmul(
                    psA[bt][:, 128 * u:128 * u + 128],
                    x3[:, k, 128 * bt:128 * bt + 128],
                    wuv[:, u, k, :],
                    start=(k == 0), stop=(k == KT - 1),
                    skip_group_check=True)

            def a_u(u, with_prior=False, ps1=None):
                # k-paced for u0 (DMA streaming); bt-major k-halves otherwise
                # so the first batch-half's psA completes early for copies
                if with_prior:
                    for k in range(KT):
                        for bt in range(2):
                            a_mm(u, bt, k)
                        if k < 8:
                            for i, (mp, m0) in enumerate([(128, 0), (32, 128)]):
                                nc.tensor.matmul(
                                    ps1[i][0:mp, :], wp13[:, k, m0:m0 + mp],
                                    x3[:, k, :], start=(k == 0 and i == 0),
                                    stop=(k == 7 and i == 1),
                                    skip_group_check=True)
                else:
                    for bt in range(2):
                        for k in range(KT):
                            a_mm(u, bt, k)

            at_n = [0]

            def at_copies_bt(u, bt):
                for rh in range(2):
                    src = psA[bt][64 * rh:64 * rh + 64,
                                  128 * u:128 * u + 128]
                    dst = lhsA_sb[0:64,
                                  128 * (4 * (2 * bt + rh) + u):
                                  128 * (4 * (2 * bt + rh) + u) + 128]
                    nc.vector.tensor_copy(dst, src)

            def at_copies(u):
                # A^T slices [64, 128] -> lhsA rows 0:64; the last block's
                # copies ride on Act so DVE is free for the tail relus
                for bt in range(2):
                    for rh in range(2):
                        src = psA[bt][64 * rh:64 * rh + 64,
                                      128 * u:128 * u + 128]
                        dst = lhsA_sb[0:64,
                                      128 * (4 * (2 * bt + rh) + u):
                                      128 * (4 * (2 * bt + rh) + u) + 128]
                        nc.vector.tensor_copy(dst, src)
                        at_n[0] += 1

            relu_n = [0]

            def fz(m, rcs):
                for rc in rcs:
                    psz = ps_z.tile([128, 512], F32, tag="pz",
                                    name=f"pz{m}_{rc}")
                    nc.tensor.matmul(
                        psz[:, :],
                        lhsA_sb[0:96,
                                128 * (4 * rc + m):128 * (4 * rc + m) + 128],
                        zind_sb[0:96, 512 * rc:512 * rc + 512],
                        start=True, stop=True)
                    dst = h_sb[m][:, 512 * rc:512 * rc + 512]
                    if RELU_ENG[4 * m + rc] == 'A':
                        nc.scalar.activation(dst, psz[:, :], RELU,
                                             bias=bias_sb[:, m:m + 1])
                    else:
                        nc.vector.tensor_scalar(dst, psz[:, :],
                                                bias_sb[:, m:m + 1], 0.0,
                                                op0=ADD, op1=MAX)
                    relu_n[0] += 1

            def l2(ts, ms):
                for t in ts:
                    for m in ms:
                        nc.tensor.matmul(
                            pso_h[t // 8][:, 32 * (t % 8):32 * (t % 8) + 32],
                            h_sb[m][:, 128 * t:128 * t + 128],
                            w23[:, m, :], start=False,
                            stop=(m == NU - 1 and t % 8 == 7),
                            skip_group_check=True)

            # ---- A(u0) + prior L1 -----------------------------------------
            # both prior m-tiles share one psum bank: the [32]-tile opens
            # via the bank's pending-zero (start=False after the big start)
            ps1t = ps_p.tile([128, 512], F32, tag="pp", name="pp1")
            ps1 = [ps1t[:, 0:BL], ps1t[:, BL:2 * BL]]
            a_u(0, with_prior=True, ps1=ps1)
            at_copies(0)
            nc.scalar.activation(h1a_sb[:], ps1[0][0:128, :], RELU,
                                 bias=bias_sb[:, 4:5])
            nc.scalar.activation(h1b_sb[:], ps1[1][0:32, :], RELU,
                                 bias=bias_sb[0:32, 5:6])

            # ---- A(u1); prior L2 ------------------------------------------
            a_u(1)
            at_copies(1)
            ps2t = ps_p.tile([128, 512], F32, tag="pp", name="pp2")
            ps2 = [ps2t[:, 0:BL], ps2t[:, BL:2 * BL]]
            for i, (mp, m0) in enumerate([(128, 0), (32, 128)]):
                ps = ps2[i]
                nc.tensor.matmul(ps[0:mp, :], wp2a_v[:, m0:m0 + mp],
                                 h1a_sb[:], start=(i == 0), stop=False,
                                 skip_group_check=True)
                nc.tensor.matmul(ps[0:mp, :], wp2b_v[:, m0:m0 + mp],
                                 h1b_sb[:], start=False, stop=(i == 1),
                                 skip_group_check=True)
            nc.scalar.activation(h2a_sb[:], ps2[0][0:128, :], RELU,
                                 bias=bias_sb[:, 6:7])
            nc.scalar.activation(h2b_sb[0:32, :], ps2[1][0:32, :], RELU,
                                 bias=bias_sb[0:32, 7:8])

            # ---- FZ(m0); prior L3 + p -------------------------------------
            fz(0, range(NRC))
            psp = ps_p.tile([128, BL], F32, tag="pp", name="ppp")
            nc.tensor.matmul(psp[0:32, :], wp3a_v[:], h2a_sb[:],
                             start=True, stop=False)
            nc.tensor.matmul(psp[0:32, :], wp3b_v[:], h2b_sb[:],
                             start=False, stop=True)
            nc.scalar.activation(p_sb[:], psp[0:32, :], COPY)

            # ---- A(u2); p transpose ---------------------------------------
            a_u(2)
            at_copies(2)
            psT = ps_p.tile([16, NT * ND], BF16, tag="pp", name="ppT")
            for c in range(NT):
                nc.tensor.transpose(psT[:, 32 * c:32 * c + 32],
                                    p_sb[0:32, 16 * c:16 * c + 16], eye32_v)
            nc.scalar.activation(pT2_sb[:], psT[:, :], COPY)

            # ---- FZ(m1); pso init with p + bias ---------------------------
            fz(1, range(NRC))
            # two pso half-tiles in separate banks: combine reads of one
            # half never anti-block L2 writes of the other
            pso_h = [ps_o.tile([128, 256], F32, tag="po", name="po0"),
                     ps_p.tile([128, 256], F32, tag="pp", name="po1")]
            for hh in range(2):
                nc.tensor.matmul(pso_h[hh][:, :], ind16_v,
                                 pT2_sb[0:16, 256 * hh:256 * hh + 256],
                                 start=True, stop=False,
                                 skip_group_check=True)

            # ---- A(u3); FZ(m2, m3); L2; quartered combine -----------------
            fz(2, range(NRC))
            for bt in range(2):
                for k in range(KT):
                    a_mm(3, bt, k)
                at_copies_bt(3, bt)
            fz(3, [0, 1])
            l2(range(0, 4), range(NU))
            fz(3, [2])

            def quartr(q):
                c0 = 128 * (q - 2)
                nc.vector.tensor_tensor(gm_sb[:, 256 + c0:256 + c0 + 128],
                                        pso_h[1][:, c0:c0 + 128],
                                        zrt_sb[:, 256 + c0:256 + c0 + 128],
                                        op=MULT)
                nc.vector.tensor_reduce(
                    out_sb[:, 4 * q:4 * q + 4].rearrange(
                        "p (t o) -> p t o", o=1),
                    gm_sb[:, 256 + c0:256 + c0 + 128].rearrange(
                        "p (t o) -> p t o", o=ND),
                    AXE_X, ADD)
                nc.sync.dma_start(out[:, 4 * q:4 * q + 4],
                                  out_sb[:, 4 * q:4 * q + 4])

            def half(q):
                c0, c1 = 256 * q, 256 * q + 256
                nc.vector.tensor_tensor(gm_sb[:, c0:c1], pso_h[q][:, :],
                                        zrt_sb[:, c0:c1], op=MULT)
                nc.vector.tensor_reduce(
                    out_sb[:, 8 * q:8 * q + 8].rearrange(
                        "p (t o) -> p t o", o=1),
                    gm_sb[:, c0:c1].rearrange("p (t o) -> p t o", o=ND),
                    AXE_X, ADD)
                nc.sync.dma_start(out[:, 8 * q:8 * q + 8],
                                  out_sb[:, 8 * q:8 * q + 8])

            l2(range(4, 8), range(NU))
            half(0)
            fz(3, [3])
            l2(range(8, 12), range(NU))
            quartr(2)
            l2(range(12, 16), range(NU))
            quartr(3)

    nc.compile()
    return nc


def _prep(x, feature, z, Wep1, bep1, Wep2, bep2, Wp1, bp1, Wp2, bp2, Wp3, bp3):
    """Host-side weight/layout prep shared across cores (layout/packing only)."""
    Wep1 = np.asarray(Wep1, np.float32)
    xfT = np.ascontiguousarray(
        np.concatenate([x, feature], axis=1).T.astype(NPDT))  # [XF, B]
    # w1 swizzle [p, (u k c)]
    w1 = np.ascontiguousarray(
        Wep1[:XF].astype(NPDT).reshape(KT, 128, NU, 128)
        .transpose(1, 2, 0, 3).reshape(128, NU * KT * 128))
    w1zq = np.ascontiguousarray(Wep1[XF:].astype(NPDT))   # [32, 512]

    ind64 = np.kron(np.eye(64, dtype=np.float32), np.ones((1, 8), np.float32))
    ind16 = np.kron(np.eye(16, dtype=np.float32), np.ones((1, 8), np.float32))

    packb = np.zeros((128, PB_COLS), NPDT)
    packb[:, PB_W2:PB_W2A] = (np.asarray(Wep2, np.float32)
                              .reshape(NU, 128, ND).transpose(1, 0, 2)
                              .reshape(128, NU * ND))
    wp2 = np.zeros((PHF, PHF), np.float32)
    wp3 = np.zeros((PHF, ND), np.float32)
    for e in range(ND):
        wp2[5 * e:5 * e + 5, 5 * e:5 * e + 5] = Wp2[e]
        wp3[5 * e:5 * e + 5, e] = np.asarray(Wp3)[e, :, 0]
    packb[:, PB_W2A:PB_W2B] = wp2[0:128]
    packb[0:32, PB_W2B:PB_W3A] = wp2[128:160]
    packb[:, PB_W3A:PB_W3B] = wp3[0:128]
    packb[0:32, PB_W3B:PB_IND] = wp3[128:160]
    packb[32, PB_W3B:PB_IND] = (np.asarray(bep2, np.float32)
                                + np.asarray(bp3, np.float32)[:, 0])
    packb[:, PB_WP1:PB_W2] = (np.asarray(Wp1, np.float32)
                               .transpose(1, 0, 2).reshape(SD, PHF)
                               .reshape(8, 128, PHF).transpose(1, 0, 2)
                               .reshape(128, 8 * PHF))
    packb[0:16, PB_IND:PB_IND + 128] = ind16
    packb[0:32, PB_EYE:PB_EYE + 32] = np.eye(32, dtype=np.float32)

    bias = np.zeros((128, 9), np.float32)
    bias[:, 0:4] = np.asarray(bep1, np.float32).reshape(NU, 128).T
    bp1f = np.asarray(bp1, np.float32).reshape(PHF)
    bp2f = np.asarray(bp2, np.float32).reshape(PHF)
    bias[:, 4] = bp1f[:128]
    bias[:32, 5] = bp1f[128:]
    bias[:, 6] = bp2f[:128]
    bias[:32, 7] = bp2f[128:]
    bias[:32, 8] = (np.asarray(bep2, np.float32)
                    + np.asarray(bp3, np.float32)[:, 0])

    shared = dict(w1=w1, w1zq=w1zq, packb=packb, bias=bias,
                  indq=np.ascontiguousarray(ind64.astype(NPDT)))
    in_maps = []
    for c in range(N_CORES):
        sl = slice(c * BL, (c + 1) * BL)
        zf = np.asarray(z)[sl].reshape(R, ND)          # r = b*8 + n
        zrt = np.ascontiguousarray(
            zf.astype(NPDT).reshape(NT, 128, ND)
            .transpose(1, 0, 2).reshape(128, NT * ND))
        m = dict(shared)
        m["xfT"] = np.ascontiguousarray(
            xfT[:, sl].reshape(KT, 128, BL).transpose(1, 0, 2)
            .reshape(128, KT * BL))
        m["zT"] = np.ascontiguousarray(zf.T.astype(NPDT))
        m["zrt"] = zrt
        in_maps.append(m)
    return in_maps


def kernel(**inputs):
    if "nc" not in _CACHE:
        _CACHE["nc"] = _build()
    nc = _CACHE["nc"]
    in_maps = _prep(**inputs)
    last_err = None
    for _attempt in range(3):
        try:
            res = run_bass_kernel_spmd(nc, in_maps, list(range(N_CORES)))
            full = np.empty((B, N_Z, 1), np.float32)
            for c in range(N_CORES):
                S = np.asarray(res.results[c]["out"])       # [128, NT]
                full[c * BL:(c + 1) * BL, :, 0] = \
                    S.T.reshape(BL, N_Z)
            return full
        except Exception as e:  # transient device/transfer hiccups
            last_err = e
            time.sleep(5.0 * (_attempt + 1))
    raise last_err
